# revision 15
# baseline (speedup 1.0000x reference)
"""TRN2 Bass kernel for nn_BasicBlock_1w8a_q (IR-Net BasicBlock, 1w8a quantized).

Strategy:
- Pure data parallel over batch: 128 images -> 8 cores x 16 images.
- All quantized values are held in a x7 integer domain: xq7 = round(7*x/T) is
  an integer in [-7,7], exactly representable in fp8(e4m3). Binarized weights
  are +/-sw with sw a power of two -> fp8 exact. Convs run on the TensorEngine
  in fp8 DoubleRow mode (K=256: 64ch x {dy,dy+1} pair-dim x {base,+2rows}
  partition replica), accumulating exactly in f32 PSUM.
- BN-as-affine + residual + hardtanh + requant chains run as DVE/ACT/GPSIMD
  elementwise passes using the magic-number (1.5*2^23) RNE rounding trick.
- Image pairs share 128-partition elementwise tiles (imgA: partitions 0-63,
  imgB: 64-127; engines support partition-shifted writes).
"""
import sys
sys.path.insert(0, '/opt/trn_rl_repo')

import numpy as np
import ml_dtypes

F8NP = ml_dtypes.float8_e4m3
f32 = np.float32

NCORES = 8
PER = 16          # images per core
PAIRS = PER // 2
C = 64
H = W = 56
PIX = H * W       # 3136
WP = W + 2        # 58 padded row width
NR = 60           # rows in fp8 buffer (58 padded rows + 2 tap-overflow rows)
MAGIC = float(f32(12582912.0))          # 1.5 * 2**23
SIGMA = float(f32(1023.0 / (576.0 * 7.0)))

_PROGRAM = None   # cached (nc) — program is input-independent


# ----------------------------------------------------------------------------
# Host-side preparation (exact f32, mirrors the jax reference semantics)
# ----------------------------------------------------------------------------

def _weight_prep(w):
    co = w.shape[0]
    wf = w.reshape(co, -1).astype(f32)
    mean = wf.mean(1, dtype=f32).astype(f32)
    std = wf.std(1, ddof=1, dtype=f32).astype(f32)
    bw = ((w - mean[:, None, None, None]) / std[:, None, None, None]).astype(f32)
    sw = (2.0 ** np.round(np.log2(np.abs(bw.reshape(co, -1)).mean(1)))).astype(f32)
    return (np.sign(bw).astype(f32) * sw[:, None, None, None]).astype(f32)


def _bn_prep(g, b, m, v):
    std = np.sqrt((v + f32(1e-5)).astype(f32)).astype(f32)
    w = (g / std).astype(f32)
    bb = (b - w * m).astype(f32)

    def quant(t, prec):
        T = f32(np.clip(max(abs(f32(t.min())), abs(f32(t.max()))), 1e-10, 255.0))
        n = f32(2 ** prec - 1)
        return ((np.round((np.clip(t, -T, T) / T).astype(f32) * n) / n).astype(f32) * T).astype(f32)

    return quant(w, 3), quant(bb, 12)


def _host_prep(x, w1, w2, g1, b1, m1, v1, g2, b2, m2, v2):
    bw1 = _weight_prep(w1)
    bw2 = _weight_prep(w2)
    qw1, qb1 = _bn_prep(g1, b1, m1, v1)
    qw2, qb2 = _bn_prep(g2, b2, m2, v2)

    T1 = f32(np.clip(max(abs(f32(x.min())), abs(f32(x.max()))), 1e-10, 255.0))
    c7 = f32(f32(7.0) / T1)
    s01 = (f32(7.0 * 576.0 / 1023.0) * qw1).astype(f32)
    s11 = (f32(7.0) * qb1).astype(f32)
    s02 = (f32(7.0 * 576.0 / 1023.0) * qw2).astype(f32)
    s12 = (f32(7.0) * qb2).astype(f32)
    assert np.all(s02 != 0), "degenerate BN scale"
    inv2 = (f32(1.0) / s02).astype(f32)
    c2t = (s12 * inv2).astype(f32)
    b_lo = np.minimum(-7 * inv2, 7 * inv2).astype(f32)
    b_hi = np.maximum(-7 * inv2, 7 * inv2).astype(f32)
    sf2 = (s02 / f32(7.0)).astype(f32)

    # per-partition scalar table [128, 9], channel tables duplicated per half
    tab = np.zeros((128, 9), f32)
    for half in (0, 1):
        s = slice(64 * half, 64 * half + 64)
        tab[s, 0] = s01
        tab[s, 1] = s11
        tab[s, 2] = inv2
        tab[s, 3] = c2t
        tab[s, 4] = b_lo
        tab[s, 5] = b_hi
        tab[s, 6] = sf2
    tab[:, 7] = MAGIC
    tab[:, 8] = c7

    # fp8 DoubleRow weight tiles: [128, 6, 2, 64]; k = conv*3 + dx
    # lhsT[p, j, co]: p<64 -> (ci=p, dy=j), p>=64 -> (ci=p-64, dy=2+j; dy=3 -> 0)
    wq = np.zeros((128, 6, 2, 64), f32)
    for ic, bw in enumerate((bw1, bw2)):
        for dx in range(3):
            for j in range(2):
                wq[0:64, ic * 3 + dx, j, :] = bw[:, :, j, dx].T
                if 2 + j <= 2:
                    wq[64:128, ic * 3 + dx, j, :] = bw[:, :, 2 + j, dx].T
    wq8 = wq.astype(F8NP)
    assert np.all(wq8.astype(f32) == wq), "weights not fp8-exact"
    return wq8, tab


# ----------------------------------------------------------------------------
# Bass program (static; all data-dependent scalars come in via the table)
# ----------------------------------------------------------------------------

def _build_program():
    global _PROGRAM
    if _PROGRAM is not None:
        return _PROGRAM

    import concourse.bacc as bacc
    import concourse.mybir as mybir
    from concourse.tile import TileContext

    import concourse.bass as bass
    F8 = mybir.dt.float8e4
    F32 = mybir.dt.float32
    ALU = mybir.AluOpType
    ACTF = mybir.ActivationFunctionType
    DR = mybir.MatmulPerfMode.DoubleRow

    nc = bacc.Bacc("TRN2", target_bir_lowering=False)

    x_in = nc.declare_dram_parameter("x", [PER, C, PIX], F32, isOutput=False)
    wq_in = nc.declare_dram_parameter("wq", [128, 6, 2, 64], F8, isOutput=False)
    tab_in = nc.declare_dram_parameter("tab", [128, 9], F32, isOutput=False)
    y_out = nc.declare_dram_parameter("y", [PER, C, PIX], F32, isOutput=True)

    with TileContext(nc) as tc:
        with tc.tile_pool(name="const", bufs=1) as constp, \
             tc.tile_pool(name="f8b", bufs=1) as f8p, \
             tc.tile_pool(name="xin", bufs=1) as xp, \
             tc.tile_pool(name="ew", bufs=12) as ewp, \
             tc.tile_pool(name="vtile", bufs=8) as vp, \
             tc.tile_pool(name="ps", bufs=4, space="PSUM") as psp:

            wt = constp.tile([128, 6, 2, 64], F8)
            tabt = constp.tile([128, 9], F32)
            nc.sync.dma_start(out=wt, in_=wq_in[:])
            nc.sync.dma_start(out=tabt, in_=tab_in[:])

            C7 = tabt[:, 8:9]
            S01 = tabt[:, 0:1]
            INV2 = tabt[:, 2:3]
            C2T = tabt[:, 3:4]
            BLO = tabt[:, 4:5]
            BHI = tabt[:, 5:6]
            SF2 = tabt[:, 6:7]
            MB = tabt[0:64, 7:8]   # magic bias AP for ACT evac ([64]-wide)
            MB128 = tabt[:, 7:8]

            # Fixed fp8 conv-input buffers (double sets for pipelining).
            # Layout [128, NR, WP]; A-variant: base on partitions 0-63,
            # replica(+2 rows) on 64-127.  B-variant: swapped halves.
            bufs = {}
            for name in ("A1", "B1", "A2", "B2"):
                bufs[name] = [f8p.tile([128, NR, WP], F8, name=f"buf{name}{i}",
                                       tag=f"buf{name}{i}")
                              for i in range(2)]
            # zero halos / tap-overflow rows once (interior+replica regions are
            # rewritten every use; everything else must stay 0)
            for name in ("A1", "B1", "A2", "B2"):
                for t in bufs[name]:
                    nc.vector.memset(t[:, :, :], 0.0)

            def quad_rhs(buf, y0, dx):
                # moving operand [128, 2, 8, 56]: (partition, j=dy-pair stride WP,
                # out-row stride WP, col stride 1) at base offset y0*WP + dx
                base = buf[:, :, :]
                part = list(base.ap[0])
                return bass.AP(tensor=base.tensor,
                               offset=base.offset + y0 * WP + dx,
                               ap=[part, [WP, 2], [WP, 8], [1, 56]])

            def conv(ic, bufA, bufB, vs):
                # one conv stage for an image pair.  A and B share lhsT
                # (both buffers use base-lo/replica-hi layout), so the
                # weight load is amortized over 4 consecutive matmuls.
                for r0 in (0, 16, 32, 48):
                    nsub = 2 if r0 < 48 else 1
                    psA = psp.tile([64, 2, 512], F32, name="psA", tag="ps")
                    psB = psp.tile([64, 2, 512], F32, name="psB", tag="ps")
                    for dx in range(3):
                        k = ic * 3 + dx
                        for ps, buf in ((psA, bufA), (psB, bufB)):
                            for sub in range(nsub):
                                nc.tensor.matmul(
                                    ps[:, sub, 0:448],
                                    wt[:, k], quad_rhs(buf, r0 + 8 * sub, dx),
                                    start=(dx == 0), stop=(dx == 2),
                                    perf_mode=DR, skip_group_check=True)
                    # evac both sub-chunks in one ACT op per image
                    nr = 8 * nsub
                    for hs, ps in ((0, psA), (1, psB)):
                        # rows r0..r0+nr may span the 24-row split of vs
                        done = 0
                        while done < nr:
                            r = r0 + done
                            vh, rr, lim = (vs[0], r, 24) if r < 24 else (vs[1], r - 24, 32)
                            take = min(nr - done, lim - rr)
                            nc.scalar.activation(
                                vh[64 * hs:64 * hs + 64, rr:rr + take, :],
                                bass.AP(tensor=ps.tensor, offset=ps.offset + (done // 8) * 512,
                                        ap=[list(ps[:, :, :].ap[0]), [512, (take + 7) // 8],
                                            [1, min(take, 8) * 56]]),
                                ACTF.Identity, bias=MB, scale=SIGMA)
                            done += take

            for p in range(PAIRS):
                bA1 = bufs["A1"][p % 2]; bB1 = bufs["B1"][p % 2]
                bA2 = bufs["A2"][p % 2]; bB2 = bufs["B2"][p % 2]

                xt = xp.tile([128, H, W], F32)
                nc.sync.dma_start(out=xt[0:64], in_=x_in[2 * p].rearrange("c (h w) -> c h w", h=H))
                nc.sync.dma_start(out=xt[64:128], in_=x_in[2 * p + 1].rearrange("c (h w) -> c h w", h=H))

                # Row-split halves (0:24, 24:56) so the dependency graph is
                # finer: the top half of each stage can run while conv/evac is
                # still producing the bottom half.
                HS = ((0, 24), (24, 56))

                # ---- stage 0 ----
                us, r1s = [], []
                for (a, b) in HS:
                    uh = ewp.tile([128, 32, W], F32, name="uh", tag="ew")
                    nc.scalar.activation(uh[:, 0:b - a, :], xt[:, a:b, :],
                                         ACTF.Identity, bias=MB128, scale=C7)
                    nc.vector.tensor_scalar(bA1[0:64, 1 + a:1 + b, 1:57],
                                            uh[0:64, 0:b - a, :], MAGIC, None,
                                            ALU.subtract)
                    nc.vector.tensor_scalar(bB1[0:64, 1 + a:1 + b, 1:57],
                                            uh[64:128, 0:b - a, :], MAGIC, None,
                                            ALU.subtract)
                    r1h = ewp.tile([128, 32, W], F32, name="r1h", tag="ew")
                    nc.vector.tensor_scalar(r1h[:, 0:b - a, :], uh[:, 0:b - a, :],
                                            MAGIC, tabt[:, 1:2],
                                            ALU.subtract, ALU.add)
                    us.append(uh)
                    r1s.append(r1h)
                # replicas (+2 rows) for the conv taps
                nc.sync.dma_start(out=bA1[64:128, 0:56, :], in_=bA1[0:64, 2:58, :])
                nc.sync.dma_start(out=bB1[64:128, 0:56, :], in_=bB1[0:64, 2:58, :])

                # ---- conv1 + evac (into half tiles) ----
                vs = [vp.tile([128, 32, W], F32, name="vh", tag="vv")
                      for _ in HS]
                conv(0, bA1, bB1, vs)

                # ---- stage 1 ----
                cls = []
                for hi, (a, b) in enumerate(HS):
                    n = b - a
                    vh = vs[hi][:, 0:n, :]
                    rh = r1s[hi][:, 0:n, :]
                    nc.vector.tensor_scalar(vh, vh, MAGIC, S01,
                                            ALU.subtract, ALU.mult)
                    nc.vector.tensor_add(out=rh, in0=vh, in1=rh)
                    nc.vector.tensor_scalar(rh, rh, MAGIC, MAGIC,
                                            ALU.add, ALU.subtract)
                    nc.vector.tensor_scalar(rh, rh, 7.0, -7.0, ALU.min, ALU.max)
                    nc.vector.tensor_scalar(bA2[0:64, 1 + a:1 + b, 1:57],
                                            r1s[hi][0:64, 0:n, :], 1.0, None,
                                            ALU.mult)
                    nc.vector.tensor_scalar(bB2[0:64, 1 + a:1 + b, 1:57],
                                            r1s[hi][64:128, 0:n, :], 1.0, None,
                                            ALU.mult)
                    cls.append(r1s[hi])
                r2s = []
                for hi, (a, b) in enumerate(HS):
                    n = b - a
                    r2h = ewp.tile([128, 32, W], F32, name="r2h", tag="ew")
                    nc.scalar.activation(r2h[:, 0:n, :], cls[hi][:, 0:n, :],
                                         ACTF.Identity, bias=C2T, scale=INV2)
                    r2s.append(r2h)
                nc.sync.dma_start(out=bA2[64:128, 0:56, :], in_=bA2[0:64, 2:58, :])
                nc.sync.dma_start(out=bB2[64:128, 0:56, :], in_=bB2[0:64, 2:58, :])

                # ---- conv2 + evac ----
                v2s = [vp.tile([128, 32, W], F32, name="v2h", tag="vv")
                       for _ in HS]
                conv(1, bA2, bB2, v2s)

                # ---- stage 2 ----
                for hi, (a, b) in enumerate(HS):
                    n = b - a
                    zh = r2s[hi][:, 0:n, :]
                    nc.vector.scalar_tensor_tensor(zh, v2s[hi][:, 0:n, :],
                                                   MAGIC, zh,
                                                   ALU.subtract, ALU.add)
                    nc.vector.tensor_scalar(zh, zh, BHI, BLO, ALU.min, ALU.max)
                    of = v2s[hi][:, 0:n, :]
                    nc.scalar.activation(of, zh, ACTF.Copy, bias=0.0,
                                         scale=SF2)
                    for img in range(2):
                        nc.sync.dma_start(
                            out=y_out[2 * p + img, :, a * W:b * W].rearrange(
                                "c (h w) -> c h w", h=n),
                            in_=v2s[hi][64 * img:64 * img + 64, 0:n, :])

    nc.finalize()
    _PROGRAM = nc
    return nc


# ----------------------------------------------------------------------------
# Entry point
# ----------------------------------------------------------------------------

def kernel(x, w1, w2, g1, b1, m1, v1, g2, b2, m2, v2, _trace=False):
    from concourse.bass_utils import run_bass_kernel_spmd

    x = np.asarray(x, f32)
    wq8, tab = _host_prep(x, np.asarray(w1, f32), np.asarray(w2, f32),
                          np.asarray(g1, f32), np.asarray(b1, f32),
                          np.asarray(m1, f32), np.asarray(v1, f32),
                          np.asarray(g2, f32), np.asarray(b2, f32),
                          np.asarray(m2, f32), np.asarray(v2, f32))
    nc = _build_program()

    xs = x.reshape(NCORES, PER, C, PIX)
    in_maps = [{"x": np.ascontiguousarray(xs[i]), "wq": wq8, "tab": tab}
               for i in range(NCORES)]
    res = run_bass_kernel_spmd(nc, in_maps, core_ids=list(range(NCORES)),
                               trace=_trace)
    y = np.stack([np.asarray(res.results[i]["y"]) for i in range(NCORES)])
    out = y.reshape(128, C, H, W).astype(f32, copy=False)
    if _trace:
        kernel.last_exec_time_ns = res.exec_time_ns
        kernel.last_results = res
    return out


# revision 16
# speedup vs baseline: 1.0111x; 1.0111x over previous
"""TRN2 Bass kernel for nn_BasicBlock_1w8a_q (IR-Net BasicBlock, 1w8a quantized).

Strategy:
- Pure data parallel over batch: 128 images -> 8 cores x 16 images.
- All quantized values are held in a x7 integer domain: xq7 = round(7*x/T) is
  an integer in [-7,7], exactly representable in fp8(e4m3). Binarized weights
  are +/-sw with sw a power of two -> fp8 exact. Convs run on the TensorEngine
  in fp8 DoubleRow mode (K=256: 64ch x {dy,dy+1} pair-dim x {base,+2rows}
  partition replica), accumulating exactly in f32 PSUM.
- BN-as-affine + residual + hardtanh + requant chains run as DVE/ACT/GPSIMD
  elementwise passes using the magic-number (1.5*2^23) RNE rounding trick.
- Image pairs share 128-partition elementwise tiles (imgA: partitions 0-63,
  imgB: 64-127; engines support partition-shifted writes).
"""
import sys
sys.path.insert(0, '/opt/trn_rl_repo')

import numpy as np
import ml_dtypes

F8NP = ml_dtypes.float8_e4m3
f32 = np.float32

NCORES = 8
PER = 16          # images per core
PAIRS = PER // 2
C = 64
H = W = 56
PIX = H * W       # 3136
WP = W + 2        # 58 padded row width
NR = 60           # rows in fp8 buffer (58 padded rows + 2 tap-overflow rows)
MAGIC = float(f32(12582912.0))          # 1.5 * 2**23
SIGMA = float(f32(1023.0 / (576.0 * 7.0)))

_PROGRAM = None   # cached (nc) — program is input-independent


# ----------------------------------------------------------------------------
# Host-side preparation (exact f32, mirrors the jax reference semantics)
# ----------------------------------------------------------------------------

def _weight_prep(w):
    co = w.shape[0]
    wf = w.reshape(co, -1).astype(f32)
    mean = wf.mean(1, dtype=f32).astype(f32)
    std = wf.std(1, ddof=1, dtype=f32).astype(f32)
    bw = ((w - mean[:, None, None, None]) / std[:, None, None, None]).astype(f32)
    sw = (2.0 ** np.round(np.log2(np.abs(bw.reshape(co, -1)).mean(1)))).astype(f32)
    return (np.sign(bw).astype(f32) * sw[:, None, None, None]).astype(f32)


def _bn_prep(g, b, m, v):
    std = np.sqrt((v + f32(1e-5)).astype(f32)).astype(f32)
    w = (g / std).astype(f32)
    bb = (b - w * m).astype(f32)

    def quant(t, prec):
        T = f32(np.clip(max(abs(f32(t.min())), abs(f32(t.max()))), 1e-10, 255.0))
        n = f32(2 ** prec - 1)
        return ((np.round((np.clip(t, -T, T) / T).astype(f32) * n) / n).astype(f32) * T).astype(f32)

    return quant(w, 3), quant(bb, 12)


def _host_prep(x, w1, w2, g1, b1, m1, v1, g2, b2, m2, v2):
    bw1 = _weight_prep(w1)
    bw2 = _weight_prep(w2)
    qw1, qb1 = _bn_prep(g1, b1, m1, v1)
    qw2, qb2 = _bn_prep(g2, b2, m2, v2)

    T1 = f32(np.clip(max(abs(f32(x.min())), abs(f32(x.max()))), 1e-10, 255.0))
    c7 = f32(f32(7.0) / T1)
    s01 = (f32(7.0 * 576.0 / 1023.0) * qw1).astype(f32)
    s11 = (f32(7.0) * qb1).astype(f32)
    s02 = (f32(7.0 * 576.0 / 1023.0) * qw2).astype(f32)
    s12 = (f32(7.0) * qb2).astype(f32)
    assert np.all(s02 != 0), "degenerate BN scale"
    inv2 = (f32(1.0) / s02).astype(f32)
    c2t = (s12 * inv2).astype(f32)
    b_lo = np.minimum(-7 * inv2, 7 * inv2).astype(f32)
    b_hi = np.maximum(-7 * inv2, 7 * inv2).astype(f32)
    sf2 = (s02 / f32(7.0)).astype(f32)

    # per-partition scalar table [128, 9], channel tables duplicated per half
    tab = np.zeros((128, 9), f32)
    for half in (0, 1):
        s = slice(64 * half, 64 * half + 64)
        tab[s, 0] = s01
        tab[s, 1] = s11
        tab[s, 2] = inv2
        tab[s, 3] = c2t
        tab[s, 4] = b_lo
        tab[s, 5] = b_hi
        tab[s, 6] = sf2
    tab[:, 7] = MAGIC
    tab[:, 8] = c7

    # fp8 DoubleRow weight tiles: [128, 6, 2, 64]; k = conv*3 + dx
    # lhsT[p, j, co]: p<64 -> (ci=p, dy=j), p>=64 -> (ci=p-64, dy=2+j; dy=3 -> 0)
    wq = np.zeros((128, 6, 2, 64), f32)
    for ic, bw in enumerate((bw1, bw2)):
        for dx in range(3):
            for j in range(2):
                wq[0:64, ic * 3 + dx, j, :] = bw[:, :, j, dx].T
                if 2 + j <= 2:
                    wq[64:128, ic * 3 + dx, j, :] = bw[:, :, 2 + j, dx].T
    wq8 = wq.astype(F8NP)
    assert np.all(wq8.astype(f32) == wq), "weights not fp8-exact"
    return wq8, tab


# ----------------------------------------------------------------------------
# Bass program (static; all data-dependent scalars come in via the table)
# ----------------------------------------------------------------------------

def _build_program():
    global _PROGRAM
    if _PROGRAM is not None:
        return _PROGRAM

    import concourse.bacc as bacc
    import concourse.mybir as mybir
    from concourse.tile import TileContext

    import concourse.bass as bass
    F8 = mybir.dt.float8e4
    F32 = mybir.dt.float32
    ALU = mybir.AluOpType
    ACTF = mybir.ActivationFunctionType
    DR = mybir.MatmulPerfMode.DoubleRow

    nc = bacc.Bacc("TRN2", target_bir_lowering=False)

    x_in = nc.declare_dram_parameter("x", [PER, C, PIX], F32, isOutput=False)
    wq_in = nc.declare_dram_parameter("wq", [128, 6, 2, 64], F8, isOutput=False)
    tab_in = nc.declare_dram_parameter("tab", [128, 9], F32, isOutput=False)
    y_out = nc.declare_dram_parameter("y", [PER, C, PIX], F32, isOutput=True)

    with TileContext(nc) as tc:
        with tc.tile_pool(name="const", bufs=1) as constp, \
             tc.tile_pool(name="f8b", bufs=1) as f8p, \
             tc.tile_pool(name="xin", bufs=2) as xp, \
             tc.tile_pool(name="ew", bufs=12) as ewp, \
             tc.tile_pool(name="vtile", bufs=7) as vp, \
             tc.tile_pool(name="ps", bufs=4, space="PSUM") as psp:

            wt = constp.tile([128, 6, 2, 64], F8)
            tabt = constp.tile([128, 9], F32)
            nc.sync.dma_start(out=wt, in_=wq_in[:])
            nc.sync.dma_start(out=tabt, in_=tab_in[:])

            C7 = tabt[:, 8:9]
            S01 = tabt[:, 0:1]
            INV2 = tabt[:, 2:3]
            C2T = tabt[:, 3:4]
            BLO = tabt[:, 4:5]
            BHI = tabt[:, 5:6]
            SF2 = tabt[:, 6:7]
            MB = tabt[0:64, 7:8]   # magic bias AP for ACT evac ([64]-wide)
            MB128 = tabt[:, 7:8]

            # Fixed fp8 conv-input buffers (double sets for pipelining).
            # Layout [128, NR, WP]; A-variant: base on partitions 0-63,
            # replica(+2 rows) on 64-127.  B-variant: swapped halves.
            bufs = {}
            for name in ("A1", "B1", "A2", "B2"):
                bufs[name] = [f8p.tile([128, NR, WP], F8, name=f"buf{name}{i}",
                                       tag=f"buf{name}{i}")
                              for i in range(2)]
            # zero halos / tap-overflow rows once (interior+replica regions are
            # rewritten every use; everything else must stay 0)
            for name in ("A1", "B1", "A2", "B2"):
                for t in bufs[name]:
                    nc.vector.memset(t[:, :, :], 0.0)

            def quad_rhs(buf, y0, dx):
                # moving operand [128, 2, 8, 56]: (partition, j=dy-pair stride WP,
                # out-row stride WP, col stride 1) at base offset y0*WP + dx
                base = buf[:, :, :]
                part = list(base.ap[0])
                return bass.AP(tensor=base.tensor,
                               offset=base.offset + y0 * WP + dx,
                               ap=[part, [WP, 2], [WP, 8], [1, 56]])

            def conv(ic, bufA, bufB, vs):
                # one conv stage for an image pair.  A and B share lhsT
                # (both buffers use base-lo/replica-hi layout), so the
                # weight load is amortized over 4 consecutive matmuls.
                for r0 in (0, 16, 32, 48):
                    nsub = 2 if r0 < 48 else 1
                    psA = psp.tile([64, 2, 512], F32, name="psA", tag="ps")
                    psB = psp.tile([64, 2, 512], F32, name="psB", tag="ps")
                    for dx in range(3):
                        k = ic * 3 + dx
                        for ps, buf in ((psA, bufA), (psB, bufB)):
                            for sub in range(nsub):
                                nc.tensor.matmul(
                                    ps[:, sub, 0:448],
                                    wt[:, k], quad_rhs(buf, r0 + 8 * sub, dx),
                                    start=(dx == 0), stop=(dx == 2),
                                    perf_mode=DR, skip_group_check=True)
                    # evac both sub-chunks in one ACT op per image
                    nr = 8 * nsub
                    for hs, ps in ((0, psA), (1, psB)):
                        # rows r0..r0+nr may span the 24-row split of vs
                        done = 0
                        while done < nr:
                            r = r0 + done
                            vh, rr, lim = (vs[0], r, 24) if r < 24 else (vs[1], r - 24, 32)
                            take = min(nr - done, lim - rr)
                            nc.scalar.activation(
                                vh[64 * hs:64 * hs + 64, rr:rr + take, :],
                                bass.AP(tensor=ps.tensor, offset=ps.offset + (done // 8) * 512,
                                        ap=[list(ps[:, :, :].ap[0]), [512, (take + 7) // 8],
                                            [1, min(take, 8) * 56]]),
                                ACTF.Identity, bias=MB, scale=SIGMA)
                            done += take

            for p in range(PAIRS):
                bA1 = bufs["A1"][p % 2]; bB1 = bufs["B1"][p % 2]
                bA2 = bufs["A2"][p % 2]; bB2 = bufs["B2"][p % 2]

                xt = xp.tile([128, H, W], F32)
                nc.sync.dma_start(out=xt[0:64], in_=x_in[2 * p].rearrange("c (h w) -> c h w", h=H))
                nc.sync.dma_start(out=xt[64:128], in_=x_in[2 * p + 1].rearrange("c (h w) -> c h w", h=H))

                # Row-split halves (0:24, 24:56) so the dependency graph is
                # finer: the top half of each stage can run while conv/evac is
                # still producing the bottom half.
                HS = ((0, 24), (24, 56))

                # ---- stage 0 ----
                us, r1s = [], []
                for (a, b) in HS:
                    uh = ewp.tile([128, 32, W], F32, name="uh", tag="ew")
                    nc.scalar.activation(uh[:, 0:b - a, :], xt[:, a:b, :],
                                         ACTF.Identity, bias=MB128, scale=C7)
                    nc.vector.tensor_scalar(bA1[0:64, 1 + a:1 + b, 1:57],
                                            uh[0:64, 0:b - a, :], MAGIC, None,
                                            ALU.subtract)
                    nc.vector.tensor_scalar(bB1[0:64, 1 + a:1 + b, 1:57],
                                            uh[64:128, 0:b - a, :], MAGIC, None,
                                            ALU.subtract)
                    r1h = ewp.tile([128, 32, W], F32, name="r1h", tag="ew")
                    nc.vector.tensor_scalar(r1h[:, 0:b - a, :], uh[:, 0:b - a, :],
                                            MAGIC, tabt[:, 1:2],
                                            ALU.subtract, ALU.add)
                    us.append(uh)
                    r1s.append(r1h)
                # replicas (+2 rows) for the conv taps
                nc.sync.dma_start(out=bA1[64:128, 0:56, :], in_=bA1[0:64, 2:58, :])
                nc.sync.dma_start(out=bB1[64:128, 0:56, :], in_=bB1[0:64, 2:58, :])

                # ---- conv1 + evac (into half tiles) ----
                vs = [vp.tile([128, 32, W], F32, name="vh", tag="vv")
                      for _ in HS]
                conv(0, bA1, bB1, vs)

                # ---- stage 1 ----
                cls = []
                for hi, (a, b) in enumerate(HS):
                    n = b - a
                    vh = vs[hi][:, 0:n, :]
                    rh = r1s[hi][:, 0:n, :]
                    nc.vector.tensor_scalar(vh, vh, MAGIC, S01,
                                            ALU.subtract, ALU.mult)
                    nc.vector.tensor_add(out=rh, in0=vh, in1=rh)
                    nc.vector.tensor_scalar(rh, rh, MAGIC, MAGIC,
                                            ALU.add, ALU.subtract)
                    nc.vector.tensor_scalar(rh, rh, 7.0, -7.0, ALU.min, ALU.max)
                    nc.vector.tensor_scalar(bA2[0:64, 1 + a:1 + b, 1:57],
                                            r1s[hi][0:64, 0:n, :], 1.0, None,
                                            ALU.mult)
                    nc.vector.tensor_scalar(bB2[0:64, 1 + a:1 + b, 1:57],
                                            r1s[hi][64:128, 0:n, :], 1.0, None,
                                            ALU.mult)
                    cls.append(r1s[hi])
                r2s = []
                for hi, (a, b) in enumerate(HS):
                    n = b - a
                    r2h = ewp.tile([128, 32, W], F32, name="r2h", tag="ew")
                    nc.scalar.activation(r2h[:, 0:n, :], cls[hi][:, 0:n, :],
                                         ACTF.Identity, bias=C2T, scale=INV2)
                    r2s.append(r2h)
                nc.sync.dma_start(out=bA2[64:128, 0:56, :], in_=bA2[0:64, 2:58, :])
                nc.sync.dma_start(out=bB2[64:128, 0:56, :], in_=bB2[0:64, 2:58, :])

                # ---- conv2 + evac ----
                v2s = [vp.tile([128, 32, W], F32, name="v2h", tag="vv")
                       for _ in HS]
                conv(1, bA2, bB2, v2s)

                # ---- stage 2 ----
                for hi, (a, b) in enumerate(HS):
                    n = b - a
                    zh = r2s[hi][:, 0:n, :]
                    nc.vector.scalar_tensor_tensor(zh, v2s[hi][:, 0:n, :],
                                                   MAGIC, zh,
                                                   ALU.subtract, ALU.add)
                    nc.vector.tensor_scalar(zh, zh, BHI, BLO, ALU.min, ALU.max)
                    of = v2s[hi][:, 0:n, :]
                    if hi == 0:
                        nc.vector.tensor_scalar(of, zh, SF2, None, ALU.mult)
                    else:
                        nc.scalar.activation(of, zh, ACTF.Copy, bias=0.0,
                                             scale=SF2)
                    for img in range(2):
                        nc.sync.dma_start(
                            out=y_out[2 * p + img, :, a * W:b * W].rearrange(
                                "c (h w) -> c h w", h=n),
                            in_=v2s[hi][64 * img:64 * img + 64, 0:n, :])

    nc.finalize()
    _PROGRAM = nc
    return nc


# ----------------------------------------------------------------------------
# Entry point
# ----------------------------------------------------------------------------

def kernel(x, w1, w2, g1, b1, m1, v1, g2, b2, m2, v2, _trace=False):
    from concourse.bass_utils import run_bass_kernel_spmd

    x = np.asarray(x, f32)
    wq8, tab = _host_prep(x, np.asarray(w1, f32), np.asarray(w2, f32),
                          np.asarray(g1, f32), np.asarray(b1, f32),
                          np.asarray(m1, f32), np.asarray(v1, f32),
                          np.asarray(g2, f32), np.asarray(b2, f32),
                          np.asarray(m2, f32), np.asarray(v2, f32))
    nc = _build_program()

    xs = x.reshape(NCORES, PER, C, PIX)
    in_maps = [{"x": np.ascontiguousarray(xs[i]), "wq": wq8, "tab": tab}
               for i in range(NCORES)]
    res = run_bass_kernel_spmd(nc, in_maps, core_ids=list(range(NCORES)),
                               trace=_trace)
    y = np.stack([np.asarray(res.results[i]["y"]) for i in range(NCORES)])
    out = y.reshape(128, C, H, W).astype(f32, copy=False)
    if _trace:
        kernel.last_exec_time_ns = res.exec_time_ns
        kernel.last_results = res
    return out


# revision 17
# speedup vs baseline: 1.0271x; 1.0159x over previous
"""TRN2 Bass kernel for nn_BasicBlock_1w8a_q (IR-Net BasicBlock, 1w8a quantized).

Strategy:
- Pure data parallel over batch: 128 images -> 8 cores x 16 images.
- All quantized values are held in a x7 integer domain: xq7 = round(7*x/T) is
  an integer in [-7,7], exactly representable in fp8(e4m3). Binarized weights
  are +/-sw with sw a power of two -> fp8 exact. Convs run on the TensorEngine
  in fp8 DoubleRow mode (K=256: 64ch x {dy,dy+1} pair-dim x {base,+2rows}
  partition replica), accumulating exactly in f32 PSUM.
- BN-as-affine + residual + hardtanh + requant chains run as DVE/ACT/GPSIMD
  elementwise passes using the magic-number (1.5*2^23) RNE rounding trick.
- Image pairs share 128-partition elementwise tiles (imgA: partitions 0-63,
  imgB: 64-127; engines support partition-shifted writes).
"""
import sys
sys.path.insert(0, '/opt/trn_rl_repo')

import numpy as np
import ml_dtypes

F8NP = ml_dtypes.float8_e4m3
f32 = np.float32

NCORES = 8
PER = 16          # images per core
PAIRS = PER // 2
C = 64
H = W = 56
PIX = H * W       # 3136
WP = W + 2        # 58 padded row width
NR = 60           # rows in fp8 buffer (58 padded rows + 2 tap-overflow rows)
MAGIC = float(f32(12582912.0))          # 1.5 * 2**23
SIGMA = float(f32(1023.0 / (576.0 * 7.0)))

_PROGRAM = None   # cached (nc) — program is input-independent


# ----------------------------------------------------------------------------
# Host-side preparation (exact f32, mirrors the jax reference semantics)
# ----------------------------------------------------------------------------

def _weight_prep(w):
    co = w.shape[0]
    wf = w.reshape(co, -1).astype(f32)
    mean = wf.mean(1, dtype=f32).astype(f32)
    std = wf.std(1, ddof=1, dtype=f32).astype(f32)
    bw = ((w - mean[:, None, None, None]) / std[:, None, None, None]).astype(f32)
    sw = (2.0 ** np.round(np.log2(np.abs(bw.reshape(co, -1)).mean(1)))).astype(f32)
    return (np.sign(bw).astype(f32) * sw[:, None, None, None]).astype(f32)


def _bn_prep(g, b, m, v):
    std = np.sqrt((v + f32(1e-5)).astype(f32)).astype(f32)
    w = (g / std).astype(f32)
    bb = (b - w * m).astype(f32)

    def quant(t, prec):
        T = f32(np.clip(max(abs(f32(t.min())), abs(f32(t.max()))), 1e-10, 255.0))
        n = f32(2 ** prec - 1)
        return ((np.round((np.clip(t, -T, T) / T).astype(f32) * n) / n).astype(f32) * T).astype(f32)

    return quant(w, 3), quant(bb, 12)


def _host_prep(x, w1, w2, g1, b1, m1, v1, g2, b2, m2, v2):
    bw1 = _weight_prep(w1)
    bw2 = _weight_prep(w2)
    qw1, qb1 = _bn_prep(g1, b1, m1, v1)
    qw2, qb2 = _bn_prep(g2, b2, m2, v2)

    T1 = f32(np.clip(max(abs(f32(x.min())), abs(f32(x.max()))), 1e-10, 255.0))
    c7 = f32(f32(7.0) / T1)
    s01 = (f32(7.0 * 576.0 / 1023.0) * qw1).astype(f32)
    s11 = (f32(7.0) * qb1).astype(f32)
    s02 = (f32(7.0 * 576.0 / 1023.0) * qw2).astype(f32)
    s12 = (f32(7.0) * qb2).astype(f32)
    assert np.all(s02 != 0), "degenerate BN scale"
    inv2 = (f32(1.0) / s02).astype(f32)
    c2t = (s12 * inv2).astype(f32)
    b_lo = np.minimum(-7 * inv2, 7 * inv2).astype(f32)
    b_hi = np.maximum(-7 * inv2, 7 * inv2).astype(f32)
    sf2 = (s02 / f32(7.0)).astype(f32)

    # per-partition scalar table [128, 9], channel tables duplicated per half
    tab = np.zeros((128, 9), f32)
    for half in (0, 1):
        s = slice(64 * half, 64 * half + 64)
        tab[s, 0] = s01
        tab[s, 1] = s11
        tab[s, 2] = inv2
        tab[s, 3] = c2t
        tab[s, 4] = b_lo
        tab[s, 5] = b_hi
        tab[s, 6] = sf2
    tab[:, 7] = MAGIC
    tab[:, 8] = c7

    # fp8 DoubleRow weight tiles: [128, 6, 2, 64]; k = conv*3 + dx
    # lhsT[p, j, co]: p<64 -> (ci=p, dy=j), p>=64 -> (ci=p-64, dy=2+j; dy=3 -> 0)
    wq = np.zeros((128, 6, 2, 64), f32)
    for ic, bw in enumerate((bw1, bw2)):
        for dx in range(3):
            for j in range(2):
                wq[0:64, ic * 3 + dx, j, :] = bw[:, :, j, dx].T
                if 2 + j <= 2:
                    wq[64:128, ic * 3 + dx, j, :] = bw[:, :, 2 + j, dx].T
    wq8 = wq.astype(F8NP)
    assert np.all(wq8.astype(f32) == wq), "weights not fp8-exact"
    return wq8, tab


# ----------------------------------------------------------------------------
# Bass program (static; all data-dependent scalars come in via the table)
# ----------------------------------------------------------------------------

def _build_program():
    global _PROGRAM
    if _PROGRAM is not None:
        return _PROGRAM

    import concourse.bacc as bacc
    import concourse.mybir as mybir
    from concourse.tile import TileContext

    import concourse.bass as bass
    F8 = mybir.dt.float8e4
    F32 = mybir.dt.float32
    ALU = mybir.AluOpType
    ACTF = mybir.ActivationFunctionType
    DR = mybir.MatmulPerfMode.DoubleRow

    nc = bacc.Bacc("TRN2", target_bir_lowering=False)

    x_in = nc.declare_dram_parameter("x", [PER, C, PIX], F32, isOutput=False)
    wq_in = nc.declare_dram_parameter("wq", [128, 6, 2, 64], F8, isOutput=False)
    tab_in = nc.declare_dram_parameter("tab", [128, 9], F32, isOutput=False)
    y_out = nc.declare_dram_parameter("y", [PER, C, PIX], F32, isOutput=True)

    with TileContext(nc) as tc:
        with tc.tile_pool(name="const", bufs=1) as constp, \
             tc.tile_pool(name="f8b", bufs=1) as f8p, \
             tc.tile_pool(name="xin", bufs=2) as xp, \
             tc.tile_pool(name="ew", bufs=12) as ewp, \
             tc.tile_pool(name="vtile", bufs=7) as vp, \
             tc.tile_pool(name="ps", bufs=4, space="PSUM") as psp:

            wt = constp.tile([128, 6, 2, 64], F8)
            tabt = constp.tile([128, 9], F32)
            nc.sync.dma_start(out=wt, in_=wq_in[:])
            nc.sync.dma_start(out=tabt, in_=tab_in[:])

            C7 = tabt[:, 8:9]
            S01 = tabt[:, 0:1]
            INV2 = tabt[:, 2:3]
            C2T = tabt[:, 3:4]
            BLO = tabt[:, 4:5]
            BHI = tabt[:, 5:6]
            SF2 = tabt[:, 6:7]
            MB = tabt[0:64, 7:8]   # magic bias AP for ACT evac ([64]-wide)
            MB128 = tabt[:, 7:8]

            # Fixed fp8 conv-input buffers (double sets for pipelining).
            # Layout [128, NR, WP]; A-variant: base on partitions 0-63,
            # replica(+2 rows) on 64-127.  B-variant: swapped halves.
            bufs = {}
            for name in ("A1", "B1", "A2", "B2"):
                bufs[name] = [f8p.tile([128, NR, WP], F8, name=f"buf{name}{i}",
                                       tag=f"buf{name}{i}")
                              for i in range(2)]
            # zero halos / tap-overflow rows once (interior+replica regions are
            # rewritten every use; everything else must stay 0)
            for name in ("A1", "B1", "A2", "B2"):
                for t in bufs[name]:
                    nc.gpsimd.memset(t[:, :, :], 0.0)

            def quad_rhs(buf, y0, dx):
                # moving operand [128, 2, 8, 56]: (partition, j=dy-pair stride WP,
                # out-row stride WP, col stride 1) at base offset y0*WP + dx
                base = buf[:, :, :]
                part = list(base.ap[0])
                return bass.AP(tensor=base.tensor,
                               offset=base.offset + y0 * WP + dx,
                               ap=[part, [WP, 2], [WP, 8], [1, 56]])

            def conv(ic, bufA, bufB, vs):
                # one conv stage for an image pair.  A and B share lhsT
                # (both buffers use base-lo/replica-hi layout), so the
                # weight load is amortized over 4 consecutive matmuls.
                for r0 in (0, 16, 32, 48):
                    nsub = 2 if r0 < 48 else 1
                    psA = psp.tile([64, 2, 512], F32, name="psA", tag="ps")
                    psB = psp.tile([64, 2, 512], F32, name="psB", tag="ps")
                    for dx in range(3):
                        k = ic * 3 + dx
                        for ps, buf in ((psA, bufA), (psB, bufB)):
                            for sub in range(nsub):
                                nc.tensor.matmul(
                                    ps[:, sub, 0:448],
                                    wt[:, k], quad_rhs(buf, r0 + 8 * sub, dx),
                                    start=(dx == 0), stop=(dx == 2),
                                    perf_mode=DR, skip_group_check=True)
                    # evac both sub-chunks in one ACT op per image
                    nr = 8 * nsub
                    for hs, ps in ((0, psA), (1, psB)):
                        # rows r0..r0+nr may span the 24-row split of vs
                        done = 0
                        while done < nr:
                            r = r0 + done
                            vh, rr, lim = (vs[0], r, 24) if r < 24 else (vs[1], r - 24, 32)
                            take = min(nr - done, lim - rr)
                            nc.scalar.activation(
                                vh[64 * hs:64 * hs + 64, rr:rr + take, :],
                                bass.AP(tensor=ps.tensor, offset=ps.offset + (done // 8) * 512,
                                        ap=[list(ps[:, :, :].ap[0]), [512, (take + 7) // 8],
                                            [1, min(take, 8) * 56]]),
                                ACTF.Identity, bias=MB, scale=SIGMA)
                            done += take

            for p in range(PAIRS):
                bA1 = bufs["A1"][p % 2]; bB1 = bufs["B1"][p % 2]
                bA2 = bufs["A2"][p % 2]; bB2 = bufs["B2"][p % 2]

                xt = xp.tile([128, H, W], F32)
                nc.sync.dma_start(out=xt[0:64], in_=x_in[2 * p].rearrange("c (h w) -> c h w", h=H))
                nc.sync.dma_start(out=xt[64:128], in_=x_in[2 * p + 1].rearrange("c (h w) -> c h w", h=H))

                # Row-split halves (0:24, 24:56) so the dependency graph is
                # finer: the top half of each stage can run while conv/evac is
                # still producing the bottom half.
                HS = ((0, 24), (24, 56))

                # ---- stage 0 ----
                us, r1s = [], []
                for (a, b) in HS:
                    uh = ewp.tile([128, 32, W], F32, name="uh", tag="ew")
                    nc.scalar.activation(uh[:, 0:b - a, :], xt[:, a:b, :],
                                         ACTF.Identity, bias=MB128, scale=C7)
                    nc.vector.tensor_scalar(bA1[0:64, 1 + a:1 + b, 1:57],
                                            uh[0:64, 0:b - a, :], MAGIC, None,
                                            ALU.subtract)
                    nc.vector.tensor_scalar(bB1[0:64, 1 + a:1 + b, 1:57],
                                            uh[64:128, 0:b - a, :], MAGIC, None,
                                            ALU.subtract)
                    r1h = ewp.tile([128, 32, W], F32, name="r1h", tag="ew")
                    nc.vector.tensor_scalar(r1h[:, 0:b - a, :], uh[:, 0:b - a, :],
                                            MAGIC, tabt[:, 1:2],
                                            ALU.subtract, ALU.add)
                    us.append(uh)
                    r1s.append(r1h)
                # replicas (+2 rows) for the conv taps
                nc.sync.dma_start(out=bA1[64:128, 0:56, :], in_=bA1[0:64, 2:58, :])
                nc.sync.dma_start(out=bB1[64:128, 0:56, :], in_=bB1[0:64, 2:58, :])

                # ---- conv1 + evac (into half tiles) ----
                vs = [vp.tile([128, 32, W], F32, name="vh", tag="vv")
                      for _ in HS]
                conv(0, bA1, bB1, vs)

                # ---- stage 1 ----
                cls = []
                for hi, (a, b) in enumerate(HS):
                    n = b - a
                    vh = vs[hi][:, 0:n, :]
                    rh = r1s[hi][:, 0:n, :]
                    nc.vector.tensor_scalar(vh, vh, MAGIC, S01,
                                            ALU.subtract, ALU.mult)
                    nc.vector.tensor_add(out=rh, in0=vh, in1=rh)
                    nc.vector.tensor_scalar(rh, rh, MAGIC, MAGIC,
                                            ALU.add, ALU.subtract)
                    nc.vector.tensor_scalar(rh, rh, 7.0, -7.0, ALU.min, ALU.max)
                    nc.vector.tensor_scalar(bA2[0:64, 1 + a:1 + b, 1:57],
                                            r1s[hi][0:64, 0:n, :], 1.0, None,
                                            ALU.mult)
                    nc.vector.tensor_scalar(bB2[0:64, 1 + a:1 + b, 1:57],
                                            r1s[hi][64:128, 0:n, :], 1.0, None,
                                            ALU.mult)
                    cls.append(r1s[hi])
                r2s = []
                for hi, (a, b) in enumerate(HS):
                    n = b - a
                    r2h = ewp.tile([128, 32, W], F32, name="r2h", tag="ew")
                    nc.scalar.activation(r2h[:, 0:n, :], cls[hi][:, 0:n, :],
                                         ACTF.Identity, bias=C2T, scale=INV2)
                    r2s.append(r2h)
                nc.sync.dma_start(out=bA2[64:128, 0:56, :], in_=bA2[0:64, 2:58, :])
                nc.sync.dma_start(out=bB2[64:128, 0:56, :], in_=bB2[0:64, 2:58, :])

                # ---- conv2 + evac ----
                v2s = [vp.tile([128, 32, W], F32, name="v2h", tag="vv")
                       for _ in HS]
                conv(1, bA2, bB2, v2s)

                # ---- stage 2 ----
                for hi, (a, b) in enumerate(HS):
                    n = b - a
                    zh = r2s[hi][:, 0:n, :]
                    nc.vector.scalar_tensor_tensor(zh, v2s[hi][:, 0:n, :],
                                                   MAGIC, zh,
                                                   ALU.subtract, ALU.add)
                    nc.vector.tensor_scalar(zh, zh, BHI, BLO, ALU.min, ALU.max)
                    of = v2s[hi][:, 0:n, :]
                    if hi == 0:
                        nc.vector.tensor_scalar(of, zh, SF2, None, ALU.mult)
                    else:
                        nc.scalar.activation(of, zh, ACTF.Copy, bias=0.0,
                                             scale=SF2)
                    for img in range(2):
                        nc.sync.dma_start(
                            out=y_out[2 * p + img, :, a * W:b * W].rearrange(
                                "c (h w) -> c h w", h=n),
                            in_=v2s[hi][64 * img:64 * img + 64, 0:n, :])

    nc.finalize()
    _PROGRAM = nc
    return nc


# ----------------------------------------------------------------------------
# Entry point
# ----------------------------------------------------------------------------

def kernel(x, w1, w2, g1, b1, m1, v1, g2, b2, m2, v2, _trace=False):
    from concourse.bass_utils import run_bass_kernel_spmd

    x = np.asarray(x, f32)
    wq8, tab = _host_prep(x, np.asarray(w1, f32), np.asarray(w2, f32),
                          np.asarray(g1, f32), np.asarray(b1, f32),
                          np.asarray(m1, f32), np.asarray(v1, f32),
                          np.asarray(g2, f32), np.asarray(b2, f32),
                          np.asarray(m2, f32), np.asarray(v2, f32))
    nc = _build_program()

    xs = x.reshape(NCORES, PER, C, PIX)
    in_maps = [{"x": np.ascontiguousarray(xs[i]), "wq": wq8, "tab": tab}
               for i in range(NCORES)]
    res = run_bass_kernel_spmd(nc, in_maps, core_ids=list(range(NCORES)),
                               trace=_trace)
    y = np.stack([np.asarray(res.results[i]["y"]) for i in range(NCORES)])
    out = y.reshape(128, C, H, W).astype(f32, copy=False)
    if _trace:
        kernel.last_exec_time_ns = res.exec_time_ns
        kernel.last_results = res
    return out
